# revision 1
# baseline (speedup 1.0000x reference)
"""Trainium2 Bass kernel for nn_ConvTransBlock (sparse window attention block).

Reference semantics (see problem statement):
  BN(batch stats) -> 1x1 qkv conv -> convDotMul(7x7, 49 logits/pixel)
  -> +rel_bias -> softmax over 49 -> windowed value aggregation
  -> 1x1 proj -> residual -> 1x1 MLP w/ exact GELU -> residual.

Sharding: 8 cores = (batch b in 0..3) x (row half in 0..1).  Each core
computes 28 output rows of one batch element for all 192 channels.  The
only cross-core communication is an AllReduce of per-channel partial
sums for the BatchNorm statistics (which are over the whole batch).

Per-core pipeline (channel-major [C partitions, pixels] layout):
  - DMA 34 halo rows of x (28 + 3 each side, zero-padded at image edge).
  - partial sum/sumsq over own 28 rows -> AllReduce -> mean/rstd.
  - xn = a*x + b fused on ACT; qkv = 1x1 conv as f32r matmuls (output
    rows host-permuted to [q_h0,k_h0,...,q_h5,k_h5,v_h0..v_h5] so each
    head's conv input [q_h;k_h] is a contiguous 64-partition slice).
  - conv logits: contract (ci=64) x (49 taps) via PSUM accumulation;
    taps paired along dx with a +1-column-skewed doubled qk buffer so
    most matmuls use the full 128-partition contract dim.
  - softmax: exp on ACT (no max subtraction; logits are O(1)), sum via
    ones-matmul, normalization deferred until after value aggregation.
  - value aggregation: for each group of <=4 taps, broadcast the
    attention rows to 128 partitions with one selector matmul (PE),
    then multiply (DVE) and accumulate (DVE+GPSIMD) against a
    x4-replicated +j-column-skewed padded V buffer (pure offset reads).
  - proj / residual / MLP (exact GELU) as f32r / bf16 matmuls.
"""

import sys

for _p in ("/opt/trn_rl_repo",):
    if _p not in sys.path:
        sys.path.insert(0, _p)

from contextlib import ExitStack

import ml_dtypes
import numpy as np

import concourse.bass as bass
import concourse.tile as tile
from concourse import bacc, mybir
from concourse.bass_utils import run_bass_kernel_spmd

F32 = mybir.dt.float32
F32R = mybir.dt.float32r
BF16 = mybir.dt.bfloat16

DIM = 192
HEADS = 6
HD = 32
WIN = 7
K = 49
B, H, W = 4, 56, 56
EPS = 1e-5
PAD = 3

NCORES = 8
ROWS = 28          # output rows per core
HROWS = 34         # rows incl. 3-row halo each side
RW = 62            # padded row width (56 + 3 + 3)
PIXI = ROWS * W    # 1568 output pixels per core
PIXH = HROWS * W   # 1904 halo pixels per core
NPIX = B * H * W   # 12544 pixels in the full batch
VREPF = HROWS * RW + 8  # 2116: vrep free size

QKV_CHUNKS = [(0, 8), (8, 8), (16, 8), (24, 8), (32, 2)]  # (row0, nrows) halo rows

RG = 4             # conv/value-agg row groups of 7 output rows
RGROWS = 7
RGPIX = RGROWS * W  # 392

# tap groups for value aggregation: per dy, dx 0..3 (4 taps) and dx 4..6 (3)
VGROUPS = [(dy, g) for dy in range(WIN) for g in range(2)]
N_GP_ADD = 8       # how many of the 13 accumulate-adds run on GPSIMD


def _f32(x):
    return np.ascontiguousarray(np.asarray(x, dtype=np.float32))


def _bf16(x):
    return np.ascontiguousarray(
        np.asarray(x, dtype=np.float32).astype(ml_dtypes.bfloat16))


def _host_consts(inp):
    """Precompute weight layouts shared by all cores."""
    qkv_w = _f32(inp["qkv_w"])      # (576, 192)
    qkv_b = _f32(inp["qkv_b"])      # (576,)
    dm_w = _f32(inp["dm_w"])        # (49, 64, 7, 7)
    dm_b = _f32(inp["dm_b"])        # (49,)
    rel_bias = _f32(inp["rel_bias"])  # (49, 6)

    # qkv output-row permutation: [q_h0, k_h0, q_h1, k_h1, ..., v_h0..v_h5]
    perm = []
    for h in range(HEADS):
        perm += list(range(32 * h, 32 * h + 32))
        perm += list(range(192 + 32 * h, 192 + 32 * h + 32))
    perm += list(range(384, 576))
    perm = np.array(perm)
    qkv_wT = _bf16(qkv_w[perm, :].T)            # (192, 576) lhsT layout
    qkv_bp = _f32(qkv_b[perm][:, None])        # (576, 1)

    # conv weights: fold q scale, pair dx taps, lhsT tiles [128, 49]
    dm_s = dm_w.copy()
    dm_s[:, :HD, :, :] *= HD ** (-0.5)
    dw = np.zeros((WIN, 4, 2, 64, K), np.float32)
    for dy in range(WIN):
        for j in range(4):
            dx0 = 2 * j
            dw[dy, j, 0] = dm_s[:, :, dy, dx0].T
            if dx0 + 1 < WIN:
                dw[dy, j, 1] = dm_s[:, :, dy, dx0 + 1].T
    dw = _bf16(dw.transpose(2, 3, 0, 1, 4).reshape(128, WIN * 4 * K))

    bias6 = _f32(dm_b[:, None] + rel_bias)     # (49, 6)

    # attention-broadcast selector tables, bf16 (49, 14*128)
    sel = np.zeros((K, len(VGROUPS), 4, 32), np.float32)
    for gi, (dy, g) in enumerate(VGROUPS):
        for j in range(4 if g == 0 else 3):
            dx = (0 if g == 0 else 4) + j
            sel[dy * WIN + dx, gi, j, :] = 1.0
    sel = _bf16(sel.reshape(K, len(VGROUPS) * 128))

    ones49 = _bf16(np.ones((K, 1), np.float32))
    ones1r = _bf16(np.ones((1, K), np.float32))

    # partition-block-sum selector for the value-agg reduce (128 -> 32)
    sel4 = np.zeros((4, 32, 32), np.float32)
    for j in range(4):
        sel4[j, np.arange(32), np.arange(32)] = 1.0
    sel4 = _bf16(sel4.reshape(128, 32))

    return {
        "qkv_wT": qkv_wT, "qkv_b": qkv_bp,
        "dw": dw, "bias6": bias6, "sel": sel, "ones49": ones49, "ones1r": ones1r, "sel4": sel4,
        "bn_gamma": _f32(inp["bn_gamma"]).reshape(DIM, 1).copy(),
        "bn_beta": _f32(inp["bn_beta"]).reshape(DIM, 1).copy(),
        "proj_wT": _bf16(np.asarray(inp["proj_w"]).T),
        "proj_b": _f32(inp["proj_b"]).reshape(DIM, 1).copy(),
        "c1_wT": _bf16(np.asarray(inp["c1_w"]).T),
        "c1_b": _f32(inp["c1_b"]).reshape(4 * DIM, 1).copy(),
        "c2_wT": _bf16(np.asarray(inp["c2_w"]).T),
        "c2_b": _f32(inp["c2_b"]).reshape(DIM, 1).copy(),
    }


def _x_slices(x):
    """Per-core (192, 1904) halo'd row slices of x, zero padded at edges."""
    out = []
    for c in range(NCORES):
        b, yh = c // 2, c % 2
        y0 = ROWS * yh
        xl = np.zeros((DIM, HROWS, W), np.float32)
        lo, hi = max(0, y0 - PAD), min(H, y0 + ROWS + PAD)
        xl[:, lo - (y0 - PAD): hi - (y0 - PAD), :] = x[b, :, lo:hi, :]
        out.append(_f32(xl.reshape(DIM, PIXH)))
    return out


def build_program():
    """Build the SPMD Bass/Tile program once."""
    nc = bacc.Bacc("TRN2", target_bir_lowering=False, debug=False,
                   enable_asserts=False, num_devices=NCORES)

    def din(name, shape, dt=F32):
        return nc.dram_tensor(name, list(shape), dt, kind="ExternalInput")

    d = {
        "xl": din("xl", (DIM, PIXH)),
        "qkv_wT": din("qkv_wT", (DIM, 3 * DIM), BF16),
        "qkv_b": din("qkv_b", (3 * DIM, 1)),
        "dw": din("dw", (128, WIN * 4 * K), BF16),
        "bias6": din("bias6", (K, HEADS)),
        "sel": din("sel", (K, len(VGROUPS) * 128), BF16),
        "ones49": din("ones49", (K, 1), BF16),
        "ones1r": din("ones1r", (1, K), BF16),
        "sel4": din("sel4", (128, 32), BF16),
        "bn_gamma": din("bn_gamma", (DIM, 1)),
        "bn_beta": din("bn_beta", (DIM, 1)),
        "proj_wT": din("proj_wT", (DIM, DIM), BF16),
        "proj_b": din("proj_b", (DIM, 1)),
        "c1_wT": din("c1_wT", (DIM, 4 * DIM), BF16),
        "c1_b": din("c1_b", (4 * DIM, 1)),
        "c2_wT": din("c2_wT", (4 * DIM, DIM), BF16),
        "c2_b": din("c2_b", (DIM, 1)),
    }
    out_d = nc.dram_tensor("out", [DIM, PIXI], F32, kind="ExternalOutput")

    with tile.TileContext(nc) as tc, ExitStack() as ctx:
        with nc.allow_low_precision(reason="bf16 matmul operands; error budget validated against reference"):
            _build_tile_kernel(ctx, tc, d, out_d)
    nc.compile()
    return nc


def _build_tile_kernel(ctx, tc, d, out_d):
    nc = tc.nc
    AF = mybir.ActivationFunctionType
    AL = mybir.AluOpType
    AX = mybir.AxisListType
    out = out_d[:]

    def r32(ap):
        return ap.bitcast(F32R)

    consts = ctx.enter_context(tc.tile_pool(name="consts", bufs=1))
    xpool = ctx.enter_context(tc.tile_pool(name="x", bufs=1))
    small = ctx.enter_context(tc.tile_pool(name="small", bufs=1))
    xnpool = ctx.enter_context(tc.tile_pool(name="xn", bufs=1))
    qkbuf = ctx.enter_context(tc.tile_pool(name="qkbuf", bufs=1))
    vbuf = ctx.enter_context(tc.tile_pool(name="vbuf", bufs=1))
    epool = ctx.enter_context(tc.tile_pool(name="e", bufs=2))
    accp = ctx.enter_context(tc.tile_pool(name="acc", bufs=2))
    prodp = ctx.enter_context(tc.tile_pool(name="prod", bufs=4))
    dram = ctx.enter_context(tc.tile_pool(name="dram", bufs=1, space="DRAM"))

    pm = ctx.enter_context(tc.tile_pool(name="pm", bufs=2, space="PSUM"))
    patt = ctx.enter_context(tc.tile_pool(name="patt", bufs=1, space="PSUM"))
    pebc = ctx.enter_context(tc.tile_pool(name="pebc", bufs=2, space="PSUM"))
    pz = ctx.enter_context(tc.tile_pool(name="pz", bufs=1, space="PSUM"))
    pred = ctx.enter_context(tc.tile_pool(name="pred", bufs=1, space="PSUM"))

    # ---- constants ------------------------------------------------------
    def load(src, shape, nm, dt=F32, pool=consts, tag=None):
        t = pool.tile(list(shape), dt, name=nm, tag=tag or nm)
        nc.sync.dma_start(t[:], src)
        return t

    w_qkv = [load(d["qkv_wT"][0:128, :], (128, 576), "wqkv0", BF16),
             load(d["qkv_wT"][128:192, :], (64, 576), "wqkv1", BF16)]
    b_qkv = [load(d["qkv_b"][128 * i:128 * i + (128 if i < 4 else 64), :],
                  (128 if i < 4 else 64, 1), f"bqkv{i}") for i in range(5)]
    w_dw = load(d["dw"][:], (128, WIN * 4 * K), "wdw", BF16)
    t_bias6 = load(d["bias6"][:], (K, HEADS), "bias6")
    t_sel = load(d["sel"][:], (K, len(VGROUPS) * 128), "sel", BF16)
    t_ones = load(d["ones49"][:], (K, 1), "ones49", BF16)
    t_ones1r = load(d["ones1r"][:], (1, K), "ones1r", BF16)
    t_sel4 = load(d["sel4"][:], (128, 32), "sel4", BF16)
    t_bng = [load(d["bn_gamma"][0:128, :], (128, 1), "bng0"),
             load(d["bn_gamma"][128:192, :], (64, 1), "bng1")]
    t_bnb = [load(d["bn_beta"][0:128, :], (128, 1), "bnb0"),
             load(d["bn_beta"][128:192, :], (64, 1), "bnb1")]
    # late-phase weights alias early-phase slots (WAR deps delay their DMA)
    w_proj = [load(d["proj_wT"][0:128, :], (128, 192), "wproj0", BF16, tag="wdw"),
              load(d["proj_wT"][128:192, :], (64, 192), "wproj1", BF16, tag="bias6")]
    t_projb = [load(d["proj_b"][0:128, :], (128, 1), "projb0"),
               load(d["proj_b"][128:192, :], (64, 1), "projb1")]
    w_c1 = [load(d["c1_wT"][0:128, :], (128, 768), "wc10", BF16, tag="wqkv0"),
            load(d["c1_wT"][128:192, :], (64, 768), "wc11", BF16, tag="wqkv1")]
    t_c1b = [load(d["c1_b"][128 * i:128 * (i + 1), :], (128, 1), f"c1b{i}")
             for i in range(6)]
    w_c2 = [load(d["c2_wT"][128 * i:128 * (i + 1), :], (128, 192), f"wc2{i}", BF16)
            for i in range(6)]
    t_c2b = [load(d["c2_b"][0:128, :], (128, 1), "c2b0"),
             load(d["c2_b"][128:192, :], (64, 1), "c2b1")]

    # ---- x load + BN stats ---------------------------------------------
    xa = xpool.tile([128, PIXH], F32, name="xa", tag="xa")
    xb = xpool.tile([64, PIXH], F32, name="xb", tag="xb")
    nc.sync.dma_start(xa[:], d["xl"][0:128, :])
    nc.sync.dma_start(xb[:], d["xl"][128:192, :])

    i0 = PAD * W  # start of the 28 owned rows within the halo pixels

    xn = [xnpool.tile([128, PIXH], BF16, name="xn0", tag="xn0"),
          xnpool.tile([64, PIXH], BF16, name="xn1", tag="xn1")]
    sq = xnpool.tile([128, PIXI], F32, name="sq", tag="sq")

    stat = small.tile([128, 2], F32, name="stat", tag="stat")
    statb = small.tile([64, 2], F32, name="statb", tag="statb")
    for t, st in ((xa, stat), (xb, statb)):
        p = t.shape[0]
        nc.vector.tensor_reduce(st[0:p, 0:1], t[0:p, i0:i0 + PIXI], AX.X, AL.add)
        nc.scalar.activation(sq[0:p, :], t[0:p, i0:i0 + PIXI], AF.Square,
                             accum_out=st[0:p, 1:2])

    cc_in = dram.tile([DIM, 2], F32, name="cc_in", tag="cc_in")
    cc_out = dram.tile([DIM, 2], F32, name="cc_out", tag="cc_out",
                       addr_space="Shared")
    nc.gpsimd.dma_start(cc_in[0:128, :], stat[:])
    nc.gpsimd.dma_start(cc_in[128:192, :], statb[:])
    nc.gpsimd.collective_compute(
        "AllReduce", AL.add, replica_groups=[list(range(NCORES))],
        ins=[cc_in[:].opt()], outs=[cc_out[:].opt()])
    gstat = small.tile([128, 2], F32, name="gstat", tag="gstat")
    gstatb = small.tile([64, 2], F32, name="gstatb", tag="gstatb")
    nc.gpsimd.dma_start(gstat[:], cc_out[0:128, :])
    nc.gpsimd.dma_start(gstatb[:], cc_out[128:192, :])

    # a = gamma * rstd ; bb = beta - mean * a
    t_a, t_bb = [], []
    for i, (gs, p) in enumerate(((gstat, 128), (gstatb, 64))):
        mean = small.tile([p, 1], F32, name=f"mean{i}", tag=f"mean{i}")
        var = small.tile([p, 1], F32, name=f"var{i}", tag=f"var{i}")
        a = small.tile([p, 1], F32, name=f"a{i}", tag=f"a{i}")
        bb = small.tile([p, 1], F32, name=f"bb{i}", tag=f"bb{i}")
        nc.scalar.mul(mean[:], gs[0:p, 0:1], 1.0 / NPIX)
        nc.scalar.mul(var[:], gs[0:p, 1:2], 1.0 / NPIX)   # E[x^2]
        nc.vector.tensor_tensor(a[:], mean[:], mean[:], AL.mult)
        nc.vector.tensor_sub(var[:], var[:], a[:])
        nc.vector.tensor_scalar_add(var[:], var[:], EPS)
        nc.scalar.activation(a[:], var[:], AF.Sqrt)
        nc.vector.reciprocal(a[:], a[:])
        nc.vector.tensor_tensor(a[:], a[:], t_bng[i][:], AL.mult)
        nc.vector.tensor_tensor(bb[:], mean[:], a[:], AL.mult)
        nc.vector.tensor_sub(bb[:], t_bnb[i][:], bb[:])
        t_a.append(a)
        t_bb.append(bb)

    for i, (t, p) in enumerate(((xa, 128), (xb, 64))):
        nc.scalar.activation(xn[i][0:p, :], t[0:p, :], AF.Identity,
                             bias=t_bb[i][:], scale=t_a[i][:])

    # ---- qkv + padded/skewed qk and v buffers ---------------------------
    # qkp[h]: [128, 34*62]; lower 64 = zero-padded qk rows of head h,
    # upper 64 = same shifted one column left (flat[i] = lower flat[i+1]).
    qkp = [qkbuf.tile([128, HROWS * RW], BF16, name=f"qkp{h}", tag=f"qkp{h}")
           for h in range(HEADS)]
    # vrep[h]: [128, 2116]; partition block j holds vpad flat[i+j] at i.
    vrep = [vbuf.tile([128, VREPF], F32, name=f"vrep{h}", tag=f"vrep{h}")
            for h in range(HEADS)]
    for h in range(HEADS):
        nc.gpsimd.memset(qkp[h][:], 0.0)
        nc.gpsimd.memset(vrep[h][:], 0.0)

    for r0, nr in QKV_CHUNKS:
        npix = nr * W
        c0 = r0 * W
        for mt in range(5):
            m0, msz = 128 * mt, (128 if mt < 4 else 64)
            ps = pm.tile([128, 448], F32, tag="pm")
            for kc in range(2):
                ksz = 128 if kc == 0 else 64
                nc.tensor.matmul(
                    ps[0:msz, 0:npix],
                    w_qkv[kc][0:ksz, m0:m0 + msz],
                    xn[kc][0:ksz, c0:c0 + npix],
                    start=(kc == 0), stop=(kc == 1))
            if mt < 3:  # qk rows: heads 2mt, 2mt+1
                for hh in range(2):
                    h = 2 * mt + hh
                    src = ps[64 * hh:64 * hh + 64, 0:npix].rearrange(
                        "p (r c) -> p r c", c=W)
                    bias = b_qkv[mt][64 * hh:64 * hh + 64, :]
                    for half in range(2):  # 0: aligned, 1: +1 col shift
                        dst = qkp[h][64 * half:64 * half + 64, :].rearrange(
                            "p (r c) -> p r c", c=RW)[
                            :, r0:r0 + nr, PAD - half:PAD - half + W]
                        nc.scalar.activation(dst, src, AF.Identity, bias=bias)
            else:  # v rows
                nheads = 4 if mt == 3 else 2
                for hh in range(nheads):
                    h = (0 if mt == 3 else 4) + hh
                    src = ps[32 * hh:32 * hh + 32, 0:npix].rearrange(
                        "p (r c) -> p r c", c=W)
                    bias = b_qkv[mt][32 * hh:32 * hh + 32, :]
                    for j in range(4):
                        base = r0 * RW + PAD - j
                        dst = vrep[h][32 * j:32 * j + 32,
                                      base:base + nr * RW].rearrange(
                            "p (r c) -> p r c", c=RW)[:, 0:nr, 0:W]
                        nc.scalar.activation(dst, src, AF.Identity, bias=bias)

    # ---- attention ------------------------------------------------------
    # out_all / x1 alias qk buffers of already-finished heads.
    out_all = [qkbuf.tile([128, PIXI], BF16, name="oa0", tag="qkp0"),
               qkbuf.tile([64, PIXI], BF16, name="oa1", tag="qkp1")]

    for h in range(HEADS):
        e_h = epool.tile([K, PIXI], BF16, tag="e")
        for rg in range(RG):
            n0 = rg * RGPIX
            ps_att = patt.tile([K, RGPIX], F32, tag="att")
            qv = qkp[h][:].rearrange("p (r c) -> p r c", c=RW)
            first = True
            for dy in range(WIN):
                for j in range(4):
                    nc.tensor.matmul(
                        ps_att[:, :],
                        w_dw[:, (dy * 4 + j) * K:(dy * 4 + j + 1) * K],
                        qv[:, rg * RGROWS + dy:rg * RGROWS + dy + RGROWS,
                           2 * j:2 * j + W],
                        start=first, stop=(dy == WIN - 1 and j == 3))
                    first = False
            nc.scalar.activation(e_h[:, n0:n0 + RGPIX], ps_att[:, :], AF.Exp,
                                 bias=t_bias6[:, h:h + 1])
            ps_z = pz.tile([1, RGPIX], F32, tag="z")
            nc.tensor.matmul(ps_z[:, :], t_ones[:], e_h[:, n0:n0 + RGPIX],
                             start=True, stop=True)
            r1 = prodp.tile([1, RGPIX], BF16, tag="r1", bufs=2)
            nc.vector.reciprocal(r1[:, :], ps_z[:, :])
            ps_rb = pz.tile([K, RGPIX], F32, tag="rbc")
            nc.tensor.matmul(ps_rb[:, :], t_ones1r[:], r1[:, :],
                             start=True, stop=True)
            nc.vector.tensor_tensor(e_h[:, n0:n0 + RGPIX],
                                    e_h[:, n0:n0 + RGPIX], ps_rb[:, :], AL.mult)

            # value aggregation: mults on DVE, adds split DVE/GPSIMD
            acc_d = accp.tile([128, RGPIX], F32, tag="acc_d")
            acc_g = accp.tile([128, RGPIX], F32, tag="acc_g")
            cast_d = accp.tile([128, RGPIX], BF16, tag="cast_d")
            cast_g = accp.tile([128, RGPIX], BF16, tag="cast_g")
            n_dve_add = 0
            for gi, (dy, g) in enumerate(VGROUPS):
                ps_ebc = pebc.tile([128, RGPIX], F32, tag="ebc")
                nc.tensor.matmul(ps_ebc[:, :],
                                 t_sel[:, gi * 128:(gi + 1) * 128],
                                 e_h[:, n0:n0 + RGPIX], start=True, stop=True)
                dx0 = 0 if g == 0 else 4
                base = (rg * RGROWS + dy) * RW + dx0
                vv = vrep[h][:, base:base + RGROWS * RW].rearrange(
                    "p (r c) -> p r c", c=RW)[:, 0:RGROWS, 0:W]
                ebv = ps_ebc[:, :].rearrange("p (r c) -> p r c", c=W)
                if gi == 0:
                    nc.vector.tensor_tensor(
                        acc_d[:, :].rearrange("p (r c) -> p r c", c=W),
                        vv, ebv, AL.mult)
                elif gi == 1:
                    nc.vector.tensor_tensor(
                        acc_g[:, :].rearrange("p (r c) -> p r c", c=W),
                        vv, ebv, AL.mult)
                else:
                    prod = prodp.tile([128, RGPIX], F32, tag="prod")
                    nc.vector.tensor_tensor(
                        prod[:, :].rearrange("p (r c) -> p r c", c=W),
                        vv, ebv, AL.mult)
                    if gi < 2 + N_GP_ADD:
                        dst = cast_g if gi == 1 + N_GP_ADD else acc_g
                        nc.gpsimd.tensor_tensor(dst[:, :], acc_g[:, :],
                                                prod[:, :], AL.add)
                    else:
                        dst = cast_d if gi == len(VGROUPS) - 1 else acc_d
                        nc.vector.tensor_tensor(dst[:, :], acc_d[:, :],
                                                prod[:, :], AL.add)
                        n_dve_add += 1
            # reduce the 4 partition blocks of both accumulators on PE
            ps_red = pred.tile([32, RGPIX], F32, tag="red")
            nc.tensor.matmul(ps_red[:, :], t_sel4[:], cast_d[:, :],
                             start=True, stop=False)
            nc.tensor.matmul(ps_red[:, :], t_sel4[:], cast_g[:, :],
                             start=False, stop=True)
            dtile, doff = (out_all[0], 32 * h) if h < 4 else (out_all[1], 32 * (h - 4))
            nc.scalar.activation(dtile[doff:doff + 32, n0:n0 + RGPIX],
                                 ps_red[:, :], AF.Identity)

    # ---- proj + residual ------------------------------------------------
    x1 = [qkbuf.tile([128, PIXI], BF16, name="x10", tag="qkp2"),
          qkbuf.tile([64, PIXI], BF16, name="x11", tag="qkp3")]
    for nt in range(RG):
        n0 = nt * RGPIX
        for mt in range(2):
            msz = 128 if mt == 0 else 64
            ps = pm.tile([128, 448], F32, tag="pm")
            for kc in range(2):
                ksz = 128 if kc == 0 else 64
                nc.tensor.matmul(ps[0:msz, 0:RGPIX],
                                 w_proj[kc][0:ksz, 128 * mt:128 * mt + msz],
                                 out_all[kc][0:ksz, n0:n0 + RGPIX],
                                 start=(kc == 0), stop=(kc == 1))
            nc.scalar.activation(x1[mt][0:msz, n0:n0 + RGPIX], ps[0:msz, 0:RGPIX],
                                 AF.Identity, bias=t_projb[mt][:])
            xsrc = xa if mt == 0 else xb
            nc.vector.tensor_tensor(
                x1[mt][0:msz, n0:n0 + RGPIX], x1[mt][0:msz, n0:n0 + RGPIX],
                xsrc[0:msz, i0 + n0:i0 + n0 + RGPIX], AL.add)

    # ---- MLP ------------------------------------------------------------
    hten = [vbuf.tile([128, PIXI], BF16, name=f"h{i}", tag=f"vrep{i}")
            for i in range(6)]
    for nt in range(RG):
        n0 = nt * RGPIX
        for mt in range(6):
            ps = pm.tile([128, 448], F32, tag="pm")
            for kc in range(2):
                ksz = 128 if kc == 0 else 64
                nc.tensor.matmul(ps[:, 0:RGPIX],
                                 w_c1[kc][0:ksz, 128 * mt:128 * (mt + 1)],
                                 x1[kc][0:ksz, n0:n0 + RGPIX],
                                 start=(kc == 0), stop=(kc == 1))
            nc.scalar.activation(hten[mt][:, n0:n0 + RGPIX], ps[:, 0:RGPIX],
                                 AF.Gelu, bias=t_c1b[mt][:])
    for nt in range(RG):
        n0 = nt * RGPIX
        for mt in range(2):
            msz = 128 if mt == 0 else 64
            ps = pm.tile([128, 448], F32, tag="pm")
            for kc in range(6):
                nc.tensor.matmul(ps[0:msz, 0:RGPIX],
                                 w_c2[kc][:, 128 * mt:128 * mt + msz],
                                 hten[kc][:, n0:n0 + RGPIX],
                                 start=(kc == 0), stop=(kc == 5))
            of = prodp.tile([128, RGPIX], F32, tag="of", bufs=2)
            nc.scalar.activation(of[0:msz, :], ps[0:msz, 0:RGPIX],
                                 AF.Identity, bias=t_c2b[mt][:])
            nc.vector.tensor_tensor(of[0:msz, :], of[0:msz, :],
                                    x1[mt][0:msz, n0:n0 + RGPIX], AL.add)
            nc.sync.dma_start(out[128 * mt:128 * mt + msz, n0:n0 + RGPIX],
                              of[0:msz, :])


_PROGRAM = None


def _get_program():
    global _PROGRAM
    if _PROGRAM is None:
        _PROGRAM = build_program()
    return _PROGRAM


def make_in_maps(inputs):
    consts = _host_consts(inputs)
    xs = _x_slices(np.asarray(inputs["x"], np.float32))
    return [{"xl": xs[c], **consts} for c in range(NCORES)]


def assemble(results):
    out = np.empty((B, DIM, H, W), np.float32)
    for c in range(NCORES):
        b, yh = c // 2, c % 2
        y0 = ROWS * yh
        out[b, :, y0:y0 + ROWS, :] = results[c]["out"].reshape(DIM, ROWS, W)
    return out


def kernel(**inputs) -> np.ndarray:
    nc = _get_program()
    in_maps = make_in_maps(inputs)
    res = run_bass_kernel_spmd(nc, in_maps, list(range(NCORES)))
    return assemble(res.results)


if __name__ == "__main__":
    import reference
    inp = {k: np.asarray(v) for k, v in reference.setup_inputs().items()}
    got = kernel(**inp)
    exp = np.asarray(reference.reference(**reference.setup_inputs()))
    err = np.abs(got - exp).max() / np.abs(exp).max()
    print("rel err:", err)



# revision 18
# speedup vs baseline: 1.0828x; 1.0828x over previous
"""Trainium2 Bass kernel for nn_ConvTransBlock (sparse window attention block).

Reference semantics (see problem statement):
  BN(batch stats) -> 1x1 qkv conv -> convDotMul(7x7, 49 logits/pixel)
  -> +rel_bias -> softmax over 49 -> windowed value aggregation
  -> 1x1 proj -> residual -> 1x1 MLP w/ exact GELU -> residual.

Sharding: 8 cores = (batch b in 0..3) x (row half in 0..1).  Each core
computes 28 output rows of one batch element for all 192 channels.  The
only cross-core communication is an AllReduce of per-channel partial
sums for the BatchNorm statistics (which are over the whole batch).

Per-core pipeline (channel-major [C partitions, pixels] layout):
  - DMA 34 halo rows of x (28 + 3 each side, zero-padded at image edge).
  - partial sum/sumsq over own 28 rows -> AllReduce -> mean/rstd.
  - xn = a*x + b fused on ACT; qkv = 1x1 conv as f32r matmuls (output
    rows host-permuted to [q_h0,k_h0,...,q_h5,k_h5,v_h0..v_h5] so each
    head's conv input [q_h;k_h] is a contiguous 64-partition slice).
  - conv logits: contract (ci=64) x (49 taps) via PSUM accumulation;
    taps paired along dx with a +1-column-skewed doubled qk buffer so
    most matmuls use the full 128-partition contract dim.
  - softmax: exp on ACT (no max subtraction; logits are O(1)), sum via
    ones-matmul, normalization deferred until after value aggregation.
  - value aggregation: for each group of <=4 taps, broadcast the
    attention rows to 128 partitions with one selector matmul (PE),
    then multiply (DVE) and accumulate (DVE+GPSIMD) against a
    x4-replicated +j-column-skewed padded V buffer (pure offset reads).
  - proj / residual / MLP (exact GELU) as f32r / bf16 matmuls.
"""

import sys

for _p in ("/opt/trn_rl_repo",):
    if _p not in sys.path:
        sys.path.insert(0, _p)

from contextlib import ExitStack

import ml_dtypes
import numpy as np

import concourse.bass as bass
import concourse.tile as tile
from concourse import bacc, mybir
from concourse.bass_utils import run_bass_kernel_spmd

F32 = mybir.dt.float32
F32R = mybir.dt.float32r
BF16 = mybir.dt.bfloat16

DIM = 192
HEADS = 6
HD = 32
WIN = 7
K = 49
B, H, W = 4, 56, 56
EPS = 1e-5
PAD = 3

NCORES = 8
ROWS = 28          # output rows per core
HROWS = 34         # rows incl. 3-row halo each side
RW = 62            # padded row width (56 + 3 + 3)
PIXI = ROWS * W    # 1568 output pixels per core
PIXH = HROWS * W   # 1904 halo pixels per core
NPIX = B * H * W   # 12544 pixels in the full batch
VREPF = HROWS * RW + 8  # 2116: vrep free size

QKV_CHUNKS = [(0, 8), (8, 8), (16, 8), (24, 8), (32, 2)]  # (row0, nrows) halo rows

RG = 4             # conv/value-agg row groups of 7 output rows
RGROWS = 7
RGPIX = RGROWS * W  # 392

# tap groups for value aggregation: per dy, dx 0..3 (4 taps) and dx 4..6 (3)
VGROUPS = [(dy, g) for dy in range(WIN) for g in range(2)]


def _f32(x):
    return np.ascontiguousarray(np.asarray(x, dtype=np.float32))


def _bf16(x):
    return np.ascontiguousarray(
        np.asarray(x, dtype=np.float32).astype(ml_dtypes.bfloat16))


def _host_consts(inp):
    """Precompute weight layouts shared by all cores."""
    qkv_w = _f32(inp["qkv_w"])      # (576, 192)
    qkv_b = _f32(inp["qkv_b"])      # (576,)
    dm_w = _f32(inp["dm_w"])        # (49, 64, 7, 7)
    dm_b = _f32(inp["dm_b"])        # (49,)
    rel_bias = _f32(inp["rel_bias"])  # (49, 6)

    # qkv output-row permutation: [q_h0, k_h0, q_h1, k_h1, ..., v_h0..v_h5]
    perm = []
    for h in range(HEADS):
        perm += list(range(32 * h, 32 * h + 32))
        perm += list(range(192 + 32 * h, 192 + 32 * h + 32))
    perm += list(range(384, 576))
    perm = np.array(perm)
    qkv_wT = _bf16(qkv_w[perm, :].T)            # (192, 576) lhsT layout
    qkv_bp = _f32(qkv_b[perm][:, None])        # (576, 1)

    # conv weights: fold q scale, pair dx taps, lhsT tiles [128, 49]
    dm_s = dm_w.copy()
    dm_s[:, :HD, :, :] *= HD ** (-0.5)
    dw = np.zeros((WIN, 4, 2, 64, K), np.float32)
    for dy in range(WIN):
        for j in range(4):
            dx0 = 2 * j
            dw[dy, j, 0] = dm_s[:, :, dy, dx0].T
            if dx0 + 1 < WIN:
                dw[dy, j, 1] = dm_s[:, :, dy, dx0 + 1].T
    dw = _bf16(dw.transpose(2, 3, 0, 1, 4).reshape(128, WIN * 4 * K))

    bias6 = _f32(dm_b[:, None] + rel_bias)     # (49, 6)

    # attention-broadcast selector tables, bf16 (49, 14*128)
    sel = np.zeros((K, len(VGROUPS), 4, 32), np.float32)
    for gi, (dy, g) in enumerate(VGROUPS):
        for j in range(4 if g == 0 else 3):
            dx = (0 if g == 0 else 4) + j
            sel[dy * WIN + dx, gi, j, :] = 1.0
    sel = _bf16(sel.reshape(K, len(VGROUPS) * 128))

    ones49 = _bf16(np.ones((K, 1), np.float32))

    # partition-block-sum selector for the value-agg reduce (128 -> 32)
    sel4 = np.zeros((4, 32, 32), np.float32)
    for j in range(4):
        sel4[j, np.arange(32), np.arange(32)] = 1.0
    sel4 = _bf16(sel4.reshape(128, 32))

    # z-gather: slice i (=4h+rg) routes sum_k e to row i of the z PSUM tile
    zsel = np.zeros((K, 24, 32), np.float32)
    for i in range(24):
        zsel[:, i, i] = 1.0
    zsel = _bf16(zsel.reshape(K, 24 * 32))

    # head/rg -> channel-block broadcast selector for 1/z normalization:
    # slice rg: [24, 192] with row 4h+rg hot on channels of head h
    sel24 = np.zeros((24, RG, HEADS, HD), np.float32)
    for h in range(HEADS):
        for rg in range(RG):
            sel24[4 * h + rg, rg, h, :] = 1.0
    sel24 = _bf16(sel24.reshape(24, RG * DIM))

    return {
        "qkv_wT": qkv_wT, "qkv_b": qkv_bp,
        "dw": dw, "bias6": bias6, "sel": sel, "ones49": ones49, "sel4": sel4,
        "zsel": zsel, "sel24": sel24,
        "bn_gamma": _f32(inp["bn_gamma"]).reshape(DIM, 1).copy(),
        "bn_beta": _f32(inp["bn_beta"]).reshape(DIM, 1).copy(),
        "proj_wT": _bf16(np.asarray(inp["proj_w"]).T),
        "proj_b": _f32(inp["proj_b"]).reshape(DIM, 1).copy(),
        "c1_wT": _bf16(np.asarray(inp["c1_w"]).T),
        "c1_b": _f32(inp["c1_b"]).reshape(4 * DIM, 1).copy(),
        "c2_wT": _bf16(np.asarray(inp["c2_w"]).T),
        "c2_b": _f32(inp["c2_b"]).reshape(DIM, 1).copy(),
    }


def _x_slices(x):
    """Per-core (192, 1904) halo'd row slices of x, zero padded at edges."""
    out = []
    for c in range(NCORES):
        b, yh = c // 2, c % 2
        y0 = ROWS * yh
        xl = np.zeros((DIM, HROWS, W), np.float32)
        lo, hi = max(0, y0 - PAD), min(H, y0 + ROWS + PAD)
        xl[:, lo - (y0 - PAD): hi - (y0 - PAD), :] = x[b, :, lo:hi, :]
        out.append(_f32(xl.reshape(DIM, PIXH)))
    return out


def build_program():
    """Build the SPMD Bass/Tile program once."""
    nc = bacc.Bacc("TRN2", target_bir_lowering=False, debug=False,
                   enable_asserts=False, num_devices=NCORES)

    def din(name, shape, dt=F32):
        return nc.dram_tensor(name, list(shape), dt, kind="ExternalInput")

    d = {
        "xl": din("xl", (DIM, PIXH)),
        "qkv_wT": din("qkv_wT", (DIM, 3 * DIM), BF16),
        "qkv_b": din("qkv_b", (3 * DIM, 1)),
        "dw": din("dw", (128, WIN * 4 * K), BF16),
        "bias6": din("bias6", (K, HEADS)),
        "sel": din("sel", (K, len(VGROUPS) * 128), BF16),
        "ones49": din("ones49", (K, 1), BF16),
        "sel4": din("sel4", (128, 32), BF16),
        "zsel": din("zsel", (K, 24 * 32), BF16),
        "sel24": din("sel24", (24, RG * DIM), BF16),
        "bn_gamma": din("bn_gamma", (DIM, 1)),
        "bn_beta": din("bn_beta", (DIM, 1)),
        "proj_wT": din("proj_wT", (DIM, DIM), BF16),
        "proj_b": din("proj_b", (DIM, 1)),
        "c1_wT": din("c1_wT", (DIM, 4 * DIM), BF16),
        "c1_b": din("c1_b", (4 * DIM, 1)),
        "c2_wT": din("c2_wT", (4 * DIM, DIM), BF16),
        "c2_b": din("c2_b", (DIM, 1)),
    }
    out_d = nc.dram_tensor("out", [DIM, PIXI], F32, kind="ExternalOutput")

    with tile.TileContext(nc) as tc, ExitStack() as ctx:
        with nc.allow_low_precision(reason="bf16 matmul operands; error budget validated against reference"):
            _build_tile_kernel(ctx, tc, d, out_d)
    nc.compile()
    return nc


def _build_tile_kernel(ctx, tc, d, out_d):
    nc = tc.nc
    AF = mybir.ActivationFunctionType
    AL = mybir.AluOpType
    AX = mybir.AxisListType
    out = out_d[:]

    def r32(ap):
        return ap.bitcast(F32R)

    consts = ctx.enter_context(tc.tile_pool(name="consts", bufs=1))
    xpool = ctx.enter_context(tc.tile_pool(name="x", bufs=1))
    small = ctx.enter_context(tc.tile_pool(name="small", bufs=1))
    xnpool = ctx.enter_context(tc.tile_pool(name="xn", bufs=1))
    qkbuf = ctx.enter_context(tc.tile_pool(name="qkbuf", bufs=1))
    vbuf = ctx.enter_context(tc.tile_pool(name="vbuf", bufs=1))
    epool = ctx.enter_context(tc.tile_pool(name="e", bufs=2))
    prodp = ctx.enter_context(tc.tile_pool(name="prod", bufs=4))
    dram = ctx.enter_context(tc.tile_pool(name="dram", bufs=1, space="DRAM"))

    pm = ctx.enter_context(tc.tile_pool(name="pm", bufs=2, space="PSUM"))
    patt = ctx.enter_context(tc.tile_pool(name="patt", bufs=1, space="PSUM"))
    pebc = ctx.enter_context(tc.tile_pool(name="pebc", bufs=2, space="PSUM"))
    pz = ctx.enter_context(tc.tile_pool(name="pz", bufs=1, space="PSUM"))
    pred = ctx.enter_context(tc.tile_pool(name="pred", bufs=2, space="PSUM"))

    # ---- constants ------------------------------------------------------
    def load(src, shape, nm, dt=F32, pool=consts, tag=None):
        t = pool.tile(list(shape), dt, name=nm, tag=tag or nm)
        nc.sync.dma_start(t[:], src)
        return t

    w_qkv = [load(d["qkv_wT"][0:128, :], (128, 576), "wqkv0", BF16),
             load(d["qkv_wT"][128:192, :], (64, 576), "wqkv1", BF16)]
    b_qkv = [load(d["qkv_b"][128 * i:128 * i + (128 if i < 4 else 64), :],
                  (128 if i < 4 else 64, 1), f"bqkv{i}") for i in range(5)]
    w_dw = load(d["dw"][:], (128, WIN * 4 * K), "wdw", BF16)
    t_bias6 = load(d["bias6"][:], (K, HEADS), "bias6")
    t_sel = load(d["sel"][:], (K, len(VGROUPS) * 128), "sel", BF16)
    t_ones = load(d["ones49"][:], (K, 1), "ones49", BF16)
    t_sel4 = load(d["sel4"][:], (128, 32), "sel4", BF16)
    t_zsel = load(d["zsel"][:], (K, 24 * 32), "zsel", BF16)
    t_sel24 = load(d["sel24"][:], (24, RG * DIM), "sel24", BF16)
    t_bng = [load(d["bn_gamma"][0:128, :], (128, 1), "bng0"),
             load(d["bn_gamma"][128:192, :], (64, 1), "bng1")]
    t_bnb = [load(d["bn_beta"][0:128, :], (128, 1), "bnb0"),
             load(d["bn_beta"][128:192, :], (64, 1), "bnb1")]
    # late-phase weights alias early-phase slots (WAR deps delay their DMA)
    w_proj = [load(d["proj_wT"][0:128, :], (128, 192), "wproj0", BF16, tag="wdw"),
              load(d["proj_wT"][128:192, :], (64, 192), "wproj1", BF16, tag="bias6")]
    t_projb = [load(d["proj_b"][0:128, :], (128, 1), "projb0"),
               load(d["proj_b"][128:192, :], (64, 1), "projb1")]
    w_c1 = [load(d["c1_wT"][0:128, :], (128, 768), "wc10", BF16, tag="wqkv0"),
            load(d["c1_wT"][128:192, :], (64, 768), "wc11", BF16, tag="wqkv1")]
    t_c1b = [load(d["c1_b"][128 * i:128 * (i + 1), :], (128, 1), f"c1b{i}")
             for i in range(6)]
    w_c2 = [load(d["c2_wT"][128 * i:128 * (i + 1), :], (128, 192), f"wc2{i}", BF16)
            for i in range(6)]
    t_c2b = [load(d["c2_b"][0:128, :], (128, 1), "c2b0"),
             load(d["c2_b"][128:192, :], (64, 1), "c2b1")]

    # ---- x load + BN stats ---------------------------------------------
    xa = xpool.tile([128, PIXH], F32, name="xa", tag="xa")
    xb = xpool.tile([64, PIXH], F32, name="xb", tag="xb")
    nc.sync.dma_start(xa[:], d["xl"][0:128, :])
    nc.sync.dma_start(xb[:], d["xl"][128:192, :])

    i0 = PAD * W  # start of the 28 owned rows within the halo pixels

    xn = [xnpool.tile([128, PIXH], BF16, name="xn0", tag="xn0"),
          xnpool.tile([64, PIXH], BF16, name="xn1", tag="xn1")]
    sq = xnpool.tile([128, PIXI], F32, name="sq", tag="sq")

    stat = small.tile([128, 2], F32, name="stat", tag="stat")
    statb = small.tile([64, 2], F32, name="statb", tag="statb")
    for t, st in ((xa, stat), (xb, statb)):
        p = t.shape[0]
        nc.vector.tensor_reduce(st[0:p, 0:1], t[0:p, i0:i0 + PIXI], AX.X, AL.add)
        nc.scalar.activation(sq[0:p, :], t[0:p, i0:i0 + PIXI], AF.Square,
                             accum_out=st[0:p, 1:2])

    cc_in = dram.tile([DIM, 2], F32, name="cc_in", tag="cc_in")
    cc_out = dram.tile([DIM, 2], F32, name="cc_out", tag="cc_out",
                       addr_space="Shared")
    nc.gpsimd.dma_start(cc_in[0:128, :], stat[:])
    nc.gpsimd.dma_start(cc_in[128:192, :], statb[:])
    nc.gpsimd.collective_compute(
        "AllReduce", AL.add, replica_groups=[list(range(NCORES))],
        ins=[cc_in[:].opt()], outs=[cc_out[:].opt()])
    gstat = small.tile([128, 2], F32, name="gstat", tag="gstat")
    gstatb = small.tile([64, 2], F32, name="gstatb", tag="gstatb")
    nc.gpsimd.dma_start(gstat[:], cc_out[0:128, :])
    nc.gpsimd.dma_start(gstatb[:], cc_out[128:192, :])

    # a = gamma * rstd ; bb = beta - mean * a
    t_a, t_bb = [], []
    for i, (gs, p) in enumerate(((gstat, 128), (gstatb, 64))):
        mean = small.tile([p, 1], F32, name=f"mean{i}", tag=f"mean{i}")
        var = small.tile([p, 1], F32, name=f"var{i}", tag=f"var{i}")
        a = small.tile([p, 1], F32, name=f"a{i}", tag=f"a{i}")
        bb = small.tile([p, 1], F32, name=f"bb{i}", tag=f"bb{i}")
        nc.scalar.mul(mean[:], gs[0:p, 0:1], 1.0 / NPIX)
        nc.scalar.mul(var[:], gs[0:p, 1:2], 1.0 / NPIX)   # E[x^2]
        nc.vector.tensor_tensor(a[:], mean[:], mean[:], AL.mult)
        nc.vector.tensor_sub(var[:], var[:], a[:])
        nc.vector.tensor_scalar_add(var[:], var[:], EPS)
        nc.scalar.activation(a[:], var[:], AF.Sqrt)
        nc.vector.reciprocal(a[:], a[:])
        nc.vector.tensor_tensor(a[:], a[:], t_bng[i][:], AL.mult)
        nc.vector.tensor_tensor(bb[:], mean[:], a[:], AL.mult)
        nc.vector.tensor_sub(bb[:], t_bnb[i][:], bb[:])
        t_a.append(a)
        t_bb.append(bb)

    for i, (t, p) in enumerate(((xa, 128), (xb, 64))):
        nc.scalar.activation(xn[i][0:p, :], t[0:p, :], AF.Identity,
                             bias=t_bb[i][:], scale=t_a[i][:])

    # ---- qkv + padded/skewed qk and v buffers ---------------------------
    # qkp[h]: [128, 34*62]; lower 64 = zero-padded qk rows of head h,
    # upper 64 = same shifted one column left (flat[i] = lower flat[i+1]).
    qkp = [qkbuf.tile([128, HROWS * RW], BF16, name=f"qkp{h}", tag=f"qkp{h}")
           for h in range(HEADS)]
    # vrep[h]: [128, 2116]; partition block j holds vpad flat[i+j] at i.
    vrep = [vbuf.tile([128, VREPF], F32, name=f"vrep{h}", tag=f"vrep{h}")
            for h in range(HEADS)]
    for h in range(HEADS):
        nc.gpsimd.memset(qkp[h][:], 0.0)
        nc.gpsimd.memset(vrep[h][:], 0.0)

    for r0, nr in QKV_CHUNKS:
        npix = nr * W
        c0 = r0 * W
        for mt in range(5):
            m0, msz = 128 * mt, (128 if mt < 4 else 64)
            ps = pm.tile([128, 448], F32, tag="pm")
            for kc in range(2):
                ksz = 128 if kc == 0 else 64
                nc.tensor.matmul(
                    ps[0:msz, 0:npix],
                    w_qkv[kc][0:ksz, m0:m0 + msz],
                    xn[kc][0:ksz, c0:c0 + npix],
                    start=(kc == 0), stop=(kc == 1))
            if mt < 3:  # qk rows: heads 2mt, 2mt+1
                for hh in range(2):
                    h = 2 * mt + hh
                    src = ps[64 * hh:64 * hh + 64, 0:npix].rearrange(
                        "p (r c) -> p r c", c=W)
                    bias = b_qkv[mt][64 * hh:64 * hh + 64, :]
                    for half in range(2):  # 0: aligned, 1: +1 col shift
                        dst = qkp[h][64 * half:64 * half + 64, :].rearrange(
                            "p (r c) -> p r c", c=RW)[
                            :, r0:r0 + nr, PAD - half:PAD - half + W]
                        nc.scalar.activation(dst, src, AF.Identity, bias=bias)
            else:  # v rows
                nheads = 4 if mt == 3 else 2
                for hh in range(nheads):
                    h = (0 if mt == 3 else 4) + hh
                    src = ps[32 * hh:32 * hh + 32, 0:npix].rearrange(
                        "p (r c) -> p r c", c=W)
                    bias = b_qkv[mt][32 * hh:32 * hh + 32, :]
                    for j in range(4):
                        base = r0 * RW + PAD - j
                        dst = vrep[h][32 * j:32 * j + 32,
                                      base:base + nr * RW].rearrange(
                            "p (r c) -> p r c", c=RW)[:, 0:nr, 0:W]
                        nc.scalar.activation(dst, src, AF.Identity, bias=bias)

    # ---- attention ------------------------------------------------------
    # out_all / x1 alias qk buffers of already-finished heads.
    out_all = [qkbuf.tile([128, PIXI], BF16, name="oa0", tag="qkp0"),
               qkbuf.tile([64, PIXI], BF16, name="oa1", tag="qkp1")]
    # softmax denominators: row 4h+rg of one PSUM tile accumulates z(h, rg)
    ps_zall = pz.tile([32, RGPIX], F32, tag="z")
    rz_all = small.tile([24, RGPIX], BF16, name="rz", tag="rz")

    for h in range(HEADS):
        e_h = epool.tile([K, PIXI], BF16, tag="e")
        for rg in range(RG):
            n0 = rg * RGPIX
            ps_att = patt.tile([K, RGPIX], F32, tag="att")
            qv = qkp[h][:].rearrange("p (r c) -> p r c", c=RW)
            first = True
            for dy in range(WIN):
                for j in range(4):
                    nc.tensor.matmul(
                        ps_att[:, :],
                        w_dw[:, (dy * 4 + j) * K:(dy * 4 + j + 1) * K],
                        qv[:, rg * RGROWS + dy:rg * RGROWS + dy + RGROWS,
                           2 * j:2 * j + W],
                        start=first, stop=(dy == WIN - 1 and j == 3))
                    first = False
            nc.scalar.activation(e_h[:, n0:n0 + RGPIX], ps_att[:, :], AF.Exp,
                                 bias=t_bias6[:, h:h + 1])
            i24 = 4 * h + rg
            nc.tensor.matmul(ps_zall[:, :], t_zsel[:, 32 * i24:32 * i24 + 32],
                             e_h[:, n0:n0 + RGPIX],
                             start=(i24 == 0), stop=(i24 == 23))

            # value aggregation: e-broadcast (PE) -> product (DVE) ->
            # 4-block reduce accumulated across all 14 groups on PE
            ps_red = pred.tile([32, RGPIX], F32, tag="red")
            for gi, (dy, g) in enumerate(VGROUPS):
                ps_ebc = pebc.tile([128, RGPIX], F32, tag="ebc")
                nc.tensor.matmul(ps_ebc[:, :],
                                 t_sel[:, gi * 128:(gi + 1) * 128],
                                 e_h[:, n0:n0 + RGPIX], start=True, stop=True)
                dx0 = 0 if g == 0 else 4
                base = (rg * RGROWS + dy) * RW + dx0
                vv = vrep[h][:, base:base + RGROWS * RW].rearrange(
                    "p (r c) -> p r c", c=RW)[:, 0:RGROWS, 0:W]
                ebv = ps_ebc[:, :].rearrange("p (r c) -> p r c", c=W)
                prod = prodp.tile([128, RGPIX], BF16, tag="prod")
                nc.vector.tensor_tensor(
                    prod[:, :].rearrange("p (r c) -> p r c", c=W),
                    vv, ebv, AL.mult)
                nc.tensor.matmul(ps_red[:, :], t_sel4[:], prod[:, :],
                                 start=(gi == 0), stop=(gi == len(VGROUPS) - 1))
            dtile, doff = (out_all[0], 32 * h) if h < 4 else (out_all[1], 32 * (h - 4))
            nc.scalar.activation(dtile[doff:doff + 32, n0:n0 + RGPIX],
                                 ps_red[:, :], AF.Identity)

    # normalize: out_all *= 1/z broadcast per head to its 32 channels
    nc.vector.reciprocal(rz_all[:, :], ps_zall[0:24, :])
    for nt in range(RG):
        n0 = nt * RGPIX
        for mt in range(2):
            msz = 128 if mt == 0 else 64
            ps_rz = pm.tile([128, 448], F32, tag="pm")
            nc.tensor.matmul(ps_rz[0:msz, 0:RGPIX],
                             t_sel24[:, nt * DIM + 128 * mt:
                                     nt * DIM + 128 * mt + msz],
                             rz_all[:, :], start=True, stop=True)
            nc.vector.tensor_tensor(out_all[mt][0:msz, n0:n0 + RGPIX],
                                    out_all[mt][0:msz, n0:n0 + RGPIX],
                                    ps_rz[0:msz, 0:RGPIX], AL.mult)

    # ---- proj + residual ------------------------------------------------
    x1 = [qkbuf.tile([128, PIXI], BF16, name="x10", tag="qkp2"),
          qkbuf.tile([64, PIXI], BF16, name="x11", tag="qkp3")]
    for nt in range(RG):
        n0 = nt * RGPIX
        for mt in range(2):
            msz = 128 if mt == 0 else 64
            ps = pm.tile([128, 448], F32, tag="pm")
            for kc in range(2):
                ksz = 128 if kc == 0 else 64
                nc.tensor.matmul(ps[0:msz, 0:RGPIX],
                                 w_proj[kc][0:ksz, 128 * mt:128 * mt + msz],
                                 out_all[kc][0:ksz, n0:n0 + RGPIX],
                                 start=(kc == 0), stop=(kc == 1))
            nc.scalar.activation(x1[mt][0:msz, n0:n0 + RGPIX], ps[0:msz, 0:RGPIX],
                                 AF.Identity, bias=t_projb[mt][:])
            xsrc = xa if mt == 0 else xb
            nc.gpsimd.tensor_tensor(
                x1[mt][0:msz, n0:n0 + RGPIX], x1[mt][0:msz, n0:n0 + RGPIX],
                xsrc[0:msz, i0 + n0:i0 + n0 + RGPIX], AL.add)

    # ---- MLP ------------------------------------------------------------
    hten = [vbuf.tile([128, PIXI], BF16, name=f"h{i}", tag=f"vrep{i}")
            for i in range(6)]
    for nt in range(RG):
        n0 = nt * RGPIX
        for mt in range(6):
            ps = pm.tile([128, 448], F32, tag="pm")
            for kc in range(2):
                ksz = 128 if kc == 0 else 64
                nc.tensor.matmul(ps[:, 0:RGPIX],
                                 w_c1[kc][0:ksz, 128 * mt:128 * (mt + 1)],
                                 x1[kc][0:ksz, n0:n0 + RGPIX],
                                 start=(kc == 0), stop=(kc == 1))
            nc.scalar.activation(hten[mt][:, n0:n0 + RGPIX], ps[:, 0:RGPIX],
                                 AF.Gelu, bias=t_c1b[mt][:])
    for nt in range(RG):
        n0 = nt * RGPIX
        for mt in range(2):
            msz = 128 if mt == 0 else 64
            ps = pm.tile([128, 448], F32, tag="pm")
            for kc in range(6):
                nc.tensor.matmul(ps[0:msz, 0:RGPIX],
                                 w_c2[kc][:, 128 * mt:128 * mt + msz],
                                 hten[kc][:, n0:n0 + RGPIX],
                                 start=(kc == 0), stop=(kc == 5))
            of = prodp.tile([128, RGPIX], F32, tag="of", bufs=2)
            nc.scalar.activation(of[0:msz, :], ps[0:msz, 0:RGPIX],
                                 AF.Identity, bias=t_c2b[mt][:])
            nc.gpsimd.tensor_tensor(of[0:msz, :], of[0:msz, :],
                                    x1[mt][0:msz, n0:n0 + RGPIX], AL.add)
            nc.sync.dma_start(out[128 * mt:128 * mt + msz, n0:n0 + RGPIX],
                              of[0:msz, :])


_PROGRAM = None


def _get_program():
    global _PROGRAM
    if _PROGRAM is None:
        _PROGRAM = build_program()
    return _PROGRAM


def make_in_maps(inputs):
    consts = _host_consts(inputs)
    xs = _x_slices(np.asarray(inputs["x"], np.float32))
    return [{"xl": xs[c], **consts} for c in range(NCORES)]


def assemble(results):
    out = np.empty((B, DIM, H, W), np.float32)
    for c in range(NCORES):
        b, yh = c // 2, c % 2
        y0 = ROWS * yh
        out[b, :, y0:y0 + ROWS, :] = results[c]["out"].reshape(DIM, ROWS, W)
    return out


def kernel(**inputs) -> np.ndarray:
    nc = _get_program()
    in_maps = make_in_maps(inputs)
    res = run_bass_kernel_spmd(nc, in_maps, list(range(NCORES)))
    return assemble(res.results)


if __name__ == "__main__":
    import reference
    inp = {k: np.asarray(v) for k, v in reference.setup_inputs().items()}
    got = kernel(**inp)
    exp = np.asarray(reference.reference(**reference.setup_inputs()))
    err = np.abs(got - exp).max() / np.abs(exp).max()
    print("rel err:", err)



# revision 30
# speedup vs baseline: 1.3324x; 1.2305x over previous
"""Trainium2 Bass kernel for nn_ConvTransBlock (sparse window attention block).

Reference semantics (see problem statement):
  BN(batch stats) -> 1x1 qkv conv -> convDotMul(7x7, 49 logits/pixel)
  -> +rel_bias -> softmax over 49 -> windowed value aggregation
  -> 1x1 proj -> residual -> 1x1 MLP w/ exact GELU -> residual.

Sharding: 8 cores = (batch b in 0..3) x (row half in 0..1).  Each core
computes 28 output rows of one batch element for all 192 channels.  The
only cross-core communication is an AllReduce of per-channel partial
sums for the BatchNorm statistics (which are over the whole batch).

Per-core pipeline (channel-major [C partitions, pixels] layout):
  - DMA 34 halo rows of x (28 + 3 each side, zero-padded at image edge).
  - partial sum/sumsq over own 28 rows -> AllReduce -> mean/rstd.
  - xn = a*x + b fused on ACT; qkv = 1x1 conv as f32r matmuls (output
    rows host-permuted to [q_h0,k_h0,...,q_h5,k_h5,v_h0..v_h5] so each
    head's conv input [q_h;k_h] is a contiguous 64-partition slice).
  - conv logits: contract (ci=64) x (49 taps) via PSUM accumulation;
    taps paired along dx with a +1-column-skewed doubled qk buffer so
    most matmuls use the full 128-partition contract dim.
  - softmax: exp on ACT (no max subtraction; logits are O(1)), sum via
    ones-matmul, normalization deferred until after value aggregation.
  - value aggregation: for each group of <=4 taps, broadcast the
    attention rows to 128 partitions with one selector matmul (PE),
    then multiply (DVE) and accumulate (DVE+GPSIMD) against a
    x4-replicated +j-column-skewed padded V buffer (pure offset reads).
  - proj / residual / MLP (exact GELU) as f32r / bf16 matmuls.
"""

import sys

for _p in ("/opt/trn_rl_repo",):
    if _p not in sys.path:
        sys.path.insert(0, _p)

from contextlib import ExitStack

import ml_dtypes
import numpy as np

import concourse.bass as bass
import concourse.tile as tile
from concourse import bacc, mybir
from concourse.bass_utils import run_bass_kernel_spmd

F32 = mybir.dt.float32
F32R = mybir.dt.float32r
BF16 = mybir.dt.bfloat16

DIM = 192
HEADS = 6
HD = 32
WIN = 7
K = 49
B, H, W = 4, 56, 56
EPS = 1e-5
PAD = 3

NCORES = 8
ROWS = 28          # output rows per core
HROWS = 34         # rows incl. 3-row halo each side
RW = 62            # padded row width (56 + 3 + 3)
PIXI = ROWS * W    # 1568 output pixels per core
PIXH = HROWS * W   # 1904 halo pixels per core
NPIX = B * H * W   # 12544 pixels in the full batch
VREPF = HROWS * RW + 8  # 2116: vrep free size

QKV_CHUNKS = [(0, 8), (8, 8), (16, 8), (24, 8), (32, 2)]  # (row0, nrows) halo rows

RG = 4             # conv/value-agg row groups of 7 output rows
RGROWS = 7
RGPIX = RGROWS * W  # 392

# tap groups for value aggregation: per dy, dx 0..3 (4 taps) and dx 4..6 (3)
VGROUPS = [(dy, g) for dy in range(WIN) for g in range(2)]
NG = len(VGROUPS)  # 14
E56 = 56           # e rows: logit k lives at row 8*(k//7)+(k%7)


def _f32(x):
    return np.ascontiguousarray(np.asarray(x, dtype=np.float32))


def _bf16(x):
    return np.ascontiguousarray(
        np.asarray(x, dtype=np.float32).astype(ml_dtypes.bfloat16))


def _host_consts(inp):
    """Precompute weight layouts shared by all cores."""
    qkv_w = _f32(inp["qkv_w"])      # (576, 192)
    qkv_b = _f32(inp["qkv_b"])      # (576,)
    dm_w = _f32(inp["dm_w"])        # (49, 64, 7, 7)
    dm_b = _f32(inp["dm_b"])        # (49,)
    rel_bias = _f32(inp["rel_bias"])  # (49, 6)

    # qkv output-row permutation: [q_h0, k_h0, q_h1, k_h1, ..., v_h0..v_h5]
    perm = []
    for h in range(HEADS):
        perm += list(range(32 * h, 32 * h + 32))
        perm += list(range(192 + 32 * h, 192 + 32 * h + 32))
    perm += list(range(384, 576))
    perm = np.array(perm)
    qkv_wT = _bf16(qkv_w[perm, :].T)            # (192, 576) lhsT layout
    qkv_bp = _f32(qkv_b[perm][:, None])        # (576, 1)

    # conv weights: fold q scale, pair dx taps, lhsT tiles [128, 128] with
    # logit k mapped to output row 8*(k//7)+(k%7); rows 8m+7 / 56.. are 0
    # (full 128 weight columns keep FWL enabled on the LDWEIGHTS).
    dm_s = dm_w.copy()
    dm_s[:, :HD, :, :] *= HD ** (-0.5)
    r8 = np.array([8 * (k // WIN) + (k % WIN) for k in range(K)])
    dw = np.zeros((WIN, 4, 2, 64, 128), np.float32)
    for dy in range(WIN):
        for j in range(4):
            dx0 = 2 * j
            dw[dy, j, 0, :, r8] = dm_s[:, :, dy, dx0]
            if dx0 + 1 < WIN:
                dw[dy, j, 1, :, r8] = dm_s[:, :, dy, dx0 + 1]
    dw = _bf16(dw.transpose(2, 3, 0, 1, 4).reshape(128, WIN * 4 * 128))

    # (56, 6) exp bias; pad rows get -60 so exp ~= 0 there
    bias6 = np.full((E56, HEADS), -60.0, np.float32)
    bias6[r8, :] = dm_b[:, None] + rel_bias
    bias6 = _f32(bias6)

    # partition-block-sum selector for the value-agg reduce (128 -> 32)
    sel4 = np.zeros((128, 128), np.float32)
    for j in range(4):
        sel4[32 * j + np.arange(32), np.arange(32)] = 1.0
    sel4 = _bf16(sel4)

    # z-gather: slice i (=4h+rg) routes sum_k e to row i of the z PSUM tile
    zsel = np.zeros((E56, 24, 128), np.float32)
    for i in range(24):
        zsel[r8, i, i] = 1.0
    zsel = _bf16(zsel.reshape(E56, 24 * 128))

    # head/rg -> channel-block broadcast selector for 1/z normalization:
    # slice (rg, mt): [24, 128] with row 4h+rg hot on channels of head h
    sel24 = np.zeros((24, RG, 2, 128), np.float32)
    for h in range(HEADS):
        mt, c0 = (0, 32 * h) if h < 4 else (1, 32 * (h - 4))
        for rg in range(RG):
            sel24[4 * h + rg, rg, mt, c0:c0 + 32] = 1.0
    sel24 = _bf16(sel24.reshape(24, RG * 256))

    return {
        "qkv_wT": qkv_wT, "qkv_b": qkv_bp,
        "dw": dw, "bias6": bias6, "sel4": sel4,
        "zsel": zsel, "sel24": sel24,
        "bn_gamma": _f32(inp["bn_gamma"]).reshape(DIM, 1).copy(),
        "bn_beta": _f32(inp["bn_beta"]).reshape(DIM, 1).copy(),
        "proj_wT": _bf16(np.asarray(inp["proj_w"]).T),
        "proj_b": _f32(inp["proj_b"]).reshape(DIM, 1).copy(),
        "c1_wT": _bf16(np.asarray(inp["c1_w"]).T),
        "c1_b": _f32(inp["c1_b"]).reshape(4 * DIM, 1).copy(),
        "c2_wT": _bf16(np.asarray(inp["c2_w"]).T),
        "c2_b": _f32(inp["c2_b"]).reshape(DIM, 1).copy(),
    }


def _x_slices(x):
    """Per-core (192, 1904) halo'd row slices of x, zero padded at edges."""
    out = []
    for c in range(NCORES):
        b, yh = c // 2, c % 2
        y0 = ROWS * yh
        xl = np.zeros((DIM, HROWS, W), np.float32)
        lo, hi = max(0, y0 - PAD), min(H, y0 + ROWS + PAD)
        xl[:, lo - (y0 - PAD): hi - (y0 - PAD), :] = x[b, :, lo:hi, :]
        out.append(_f32(xl.reshape(DIM, PIXH)))
    return out


def build_program():
    """Build the SPMD Bass/Tile program once."""
    nc = bacc.Bacc("TRN2", target_bir_lowering=False, debug=False,
                   enable_asserts=False, num_devices=NCORES)

    def din(name, shape, dt=F32):
        return nc.dram_tensor(name, list(shape), dt, kind="ExternalInput")

    d = {
        "xl": din("xl", (DIM, PIXH)),
        "qkv_wT": din("qkv_wT", (DIM, 3 * DIM), BF16),
        "qkv_b": din("qkv_b", (3 * DIM, 1)),
        "dw": din("dw", (128, WIN * 4 * 128), BF16),
        "bias6": din("bias6", (E56, HEADS)),
        "sel4": din("sel4", (128, 128), BF16),
        "zsel": din("zsel", (E56, 24 * 128), BF16),
        "sel24": din("sel24", (24, RG * 256), BF16),
        "bn_gamma": din("bn_gamma", (DIM, 1)),
        "bn_beta": din("bn_beta", (DIM, 1)),
        "proj_wT": din("proj_wT", (DIM, DIM), BF16),
        "proj_b": din("proj_b", (DIM, 1)),
        "c1_wT": din("c1_wT", (DIM, 4 * DIM), BF16),
        "c1_b": din("c1_b", (4 * DIM, 1)),
        "c2_wT": din("c2_wT", (4 * DIM, DIM), BF16),
        "c2_b": din("c2_b", (DIM, 1)),
    }
    out_d = nc.dram_tensor("out", [DIM, PIXI], F32, kind="ExternalOutput")

    with tile.TileContext(nc) as tc, ExitStack() as ctx:
        with nc.allow_low_precision(reason="bf16 matmul operands; error budget validated against reference"):
            _build_tile_kernel(ctx, tc, d, out_d)
    nc.compile()
    return nc


def _build_tile_kernel(ctx, tc, d, out_d):
    nc = tc.nc
    AF = mybir.ActivationFunctionType
    AL = mybir.AluOpType
    AX = mybir.AxisListType
    out = out_d[:]

    def r32(ap):
        return ap.bitcast(F32R)

    consts = ctx.enter_context(tc.tile_pool(name="consts", bufs=1))
    xpool = ctx.enter_context(tc.tile_pool(name="x", bufs=1))
    small = ctx.enter_context(tc.tile_pool(name="small", bufs=1))
    xnpool = ctx.enter_context(tc.tile_pool(name="xn", bufs=1))
    qkbuf = ctx.enter_context(tc.tile_pool(name="qkbuf", bufs=1))
    vbuf = ctx.enter_context(tc.tile_pool(name="vbuf", bufs=1))
    epool = ctx.enter_context(tc.tile_pool(name="e", bufs=2))
    prodp = ctx.enter_context(tc.tile_pool(name="prod", bufs=4))
    ebcp = ctx.enter_context(tc.tile_pool(name="ebc", bufs=2))
    dram = ctx.enter_context(tc.tile_pool(name="dram", bufs=1, space="DRAM"))

    pm = ctx.enter_context(tc.tile_pool(name="pm", bufs=2, space="PSUM"))
    patt = ctx.enter_context(tc.tile_pool(name="patt", bufs=2, space="PSUM"))
    pz = ctx.enter_context(tc.tile_pool(name="pz", bufs=1, space="PSUM"))
    pred = ctx.enter_context(tc.tile_pool(name="pred", bufs=2, space="PSUM"))

    # ---- constants ------------------------------------------------------
    def load(src, shape, nm, dt=F32, pool=consts, tag=None):
        t = pool.tile(list(shape), dt, name=nm, tag=tag or nm)
        nc.sync.dma_start(t[:], src)
        return t

    w_qkv = [load(d["qkv_wT"][0:128, :], (128, 576), "wqkv0", BF16),
             load(d["qkv_wT"][128:192, :], (64, 576), "wqkv1", BF16)]
    b_qkv = [load(d["qkv_b"][128 * i:128 * i + (128 if i < 4 else 64), :],
                  (128 if i < 4 else 64, 1), f"bqkv{i}") for i in range(5)]
    w_dw = load(d["dw"][:], (128, WIN * 4 * 128), "wdw", BF16)
    t_bias6 = load(d["bias6"][:], (E56, HEADS), "bias6")
    t_sel4 = load(d["sel4"][:], (128, 128), "sel4", BF16)
    t_zsel = load(d["zsel"][:], (E56, 24 * 128), "zsel", BF16)
    t_sel24 = load(d["sel24"][:], (24, RG * 256), "sel24", BF16)
    t_bng = [load(d["bn_gamma"][0:128, :], (128, 1), "bng0"),
             load(d["bn_gamma"][128:192, :], (64, 1), "bng1")]
    t_bnb = [load(d["bn_beta"][0:128, :], (128, 1), "bnb0"),
             load(d["bn_beta"][128:192, :], (64, 1), "bnb1")]
    # late-phase weights alias early-phase slots (WAR deps delay their DMA)
    w_proj = [load(d["proj_wT"][0:128, :], (128, 192), "wproj0", BF16, tag="wdw"),
              load(d["proj_wT"][128:192, :], (64, 192), "wproj1", BF16, tag="bias6")]
    t_projb = [load(d["proj_b"][0:128, :], (128, 1), "projb0"),
               load(d["proj_b"][128:192, :], (64, 1), "projb1")]
    w_c1 = [load(d["c1_wT"][0:128, :], (128, 768), "wc10", BF16, tag="wqkv0"),
            load(d["c1_wT"][128:192, :], (64, 768), "wc11", BF16, tag="wqkv1")]
    t_c1b = [load(d["c1_b"][128 * i:128 * (i + 1), :], (128, 1), f"c1b{i}")
             for i in range(6)]
    w_c2 = [load(d["c2_wT"][128 * i:128 * (i + 1), :], (128, 192), f"wc2{i}", BF16)
            for i in range(6)]
    t_c2b = [load(d["c2_b"][0:128, :], (128, 1), "c2b0"),
             load(d["c2_b"][128:192, :], (64, 1), "c2b1")]

    # ---- x load + BN stats ---------------------------------------------
    xa = xpool.tile([128, PIXH], F32, name="xa", tag="xa")
    xb = xpool.tile([64, PIXH], F32, name="xb", tag="xb")
    nc.sync.dma_start(xa[:], d["xl"][0:128, :])
    nc.sync.dma_start(xb[:], d["xl"][128:192, :])

    i0 = PAD * W  # start of the 28 owned rows within the halo pixels

    xn = [xnpool.tile([128, PIXH], BF16, name="xn0", tag="xn0"),
          xnpool.tile([64, PIXH], BF16, name="xn1", tag="xn1")]
    sq = xnpool.tile([128, PIXI], F32, name="sq", tag="sq")

    stat = small.tile([128, 2], F32, name="stat", tag="stat")
    statb = small.tile([64, 2], F32, name="statb", tag="statb")
    for t, st in ((xa, stat), (xb, statb)):
        p = t.shape[0]
        nc.vector.tensor_reduce(st[0:p, 0:1], t[0:p, i0:i0 + PIXI], AX.X, AL.add)
        nc.scalar.activation(sq[0:p, :], t[0:p, i0:i0 + PIXI], AF.Square,
                             accum_out=st[0:p, 1:2])

    cc_in = dram.tile([DIM, 2], F32, name="cc_in", tag="cc_in")
    cc_out = dram.tile([DIM, 2], F32, name="cc_out", tag="cc_out",
                       addr_space="Shared")
    nc.gpsimd.dma_start(cc_in[0:128, :], stat[:])
    nc.gpsimd.dma_start(cc_in[128:192, :], statb[:])
    nc.gpsimd.collective_compute(
        "AllReduce", AL.add, replica_groups=[list(range(NCORES))],
        ins=[cc_in[:].opt()], outs=[cc_out[:].opt()])
    gstat = small.tile([128, 2], F32, name="gstat", tag="gstat")
    gstatb = small.tile([64, 2], F32, name="gstatb", tag="gstatb")
    nc.gpsimd.dma_start(gstat[:], cc_out[0:128, :])
    nc.gpsimd.dma_start(gstatb[:], cc_out[128:192, :])

    # a = gamma * rstd ; bb = beta - mean * a
    t_a, t_bb = [], []
    for i, (gs, p) in enumerate(((gstat, 128), (gstatb, 64))):
        mean = small.tile([p, 1], F32, name=f"mean{i}", tag=f"mean{i}")
        var = small.tile([p, 1], F32, name=f"var{i}", tag=f"var{i}")
        a = small.tile([p, 1], F32, name=f"a{i}", tag=f"a{i}")
        bb = small.tile([p, 1], F32, name=f"bb{i}", tag=f"bb{i}")
        nc.scalar.mul(mean[:], gs[0:p, 0:1], 1.0 / NPIX)
        nc.scalar.mul(var[:], gs[0:p, 1:2], 1.0 / NPIX)   # E[x^2]
        nc.vector.tensor_tensor(a[:], mean[:], mean[:], AL.mult)
        nc.vector.tensor_sub(var[:], var[:], a[:])
        nc.vector.tensor_scalar_add(var[:], var[:], EPS)
        nc.scalar.activation(a[:], var[:], AF.Sqrt)
        nc.vector.reciprocal(a[:], a[:])
        nc.vector.tensor_tensor(a[:], a[:], t_bng[i][:], AL.mult)
        nc.vector.tensor_tensor(bb[:], mean[:], a[:], AL.mult)
        nc.vector.tensor_sub(bb[:], t_bnb[i][:], bb[:])
        t_a.append(a)
        t_bb.append(bb)

    for i, (t, p) in enumerate(((xa, 128), (xb, 64))):
        nc.scalar.activation(xn[i][0:p, :], t[0:p, :], AF.Identity,
                             bias=t_bb[i][:], scale=t_a[i][:])

    # ---- qkv + padded/skewed qk and v buffers ---------------------------
    # qkp[h]: [128, 34*62]; lower 64 = zero-padded qk rows of head h,
    # upper 64 = same shifted one column left (flat[i] = lower flat[i+1]).
    qkp = [qkbuf.tile([128, HROWS * RW], BF16, name=f"qkp{h}", tag=f"qkp{h}")
           for h in range(HEADS)]
    # vrep[h]: [128, 2116]; partition block j holds vpad flat[i+j] at i.
    vrep = [vbuf.tile([128, VREPF], BF16, name=f"vrep{h}", tag=f"vrep{h}")
            for h in range(HEADS)]
    for h in range(HEADS):
        nc.gpsimd.memset(qkp[h][:], 0.0)
        nc.gpsimd.memset(vrep[h][:], 0.0)

    for r0, nr in QKV_CHUNKS:
        npix = nr * W
        c0 = r0 * W
        for mt in range(5):
            m0, msz = 128 * mt, (128 if mt < 4 else 64)
            ps = pm.tile([128, 448], F32, tag="pm")
            for kc in range(2):
                ksz = 128 if kc == 0 else 64
                nc.tensor.matmul(
                    ps[0:msz, 0:npix],
                    w_qkv[kc][0:ksz, m0:m0 + msz],
                    xn[kc][0:ksz, c0:c0 + npix],
                    start=(kc == 0), stop=(kc == 1))
            if mt < 3:  # qk rows: heads 2mt, 2mt+1
                for hh in range(2):
                    h = 2 * mt + hh
                    src = ps[64 * hh:64 * hh + 64, 0:npix].rearrange(
                        "p (r c) -> p r c", c=W)
                    bias = b_qkv[mt][64 * hh:64 * hh + 64, :]
                    for half in range(2):  # 0: aligned, 1: +1 col shift
                        dst = qkp[h][64 * half:64 * half + 64, :].rearrange(
                            "p (r c) -> p r c", c=RW)[
                            :, r0:r0 + nr, PAD - half:PAD - half + W]
                        nc.scalar.activation(dst, src, AF.Identity, bias=bias)
            else:  # v rows
                nheads = 4 if mt == 3 else 2
                for hh in range(nheads):
                    h = (0 if mt == 3 else 4) + hh
                    src = ps[32 * hh:32 * hh + 32, 0:npix].rearrange(
                        "p (r c) -> p r c", c=W)
                    bias = b_qkv[mt][32 * hh:32 * hh + 32, :]
                    for j in range(4):
                        base = r0 * RW + PAD - j
                        dst = vrep[h][32 * j:32 * j + 32,
                                      base:base + nr * RW].rearrange(
                            "p (r c) -> p r c", c=RW)[:, 0:nr, 0:W]
                        nc.scalar.activation(dst, src, AF.Identity, bias=bias)

    # ---- attention ------------------------------------------------------
    # out_all / x1 alias qk buffers of already-finished heads.
    out_all = [qkbuf.tile([128, PIXI], BF16, name="oa0", tag="qkp0"),
               qkbuf.tile([64, PIXI], BF16, name="oa1", tag="qkp1")]
    # softmax denominators: row 4h+rg of one PSUM tile accumulates z(h, rg)
    ps_zall = pz.tile([128, RGPIX], F32, tag="z")
    rz_all = small.tile([24, RGPIX], BF16, name="rz", tag="rz")

    def emit_agg(h, rg, ebc, n0):
        """Products (DVE/Pool, all-SBUF bf16) + 4-block-reduce accumulated
        on PE; runs one (h, rg) behind the conv stage so PE never stalls."""
        ps_red = pred.tile([128, RGPIX], F32, tag="red")
        for gi, (dy, g) in enumerate(VGROUPS):
            dx0 = 0 if g == 0 else 4
            base = (rg * RGROWS + dy) * RW + dx0
            vv = vrep[h][:, base:base + RGROWS * RW].rearrange(
                "p (r c) -> p r c", c=RW)[:, 0:RGROWS, 0:W]
            ebv = ebc[:, gi * RGPIX:(gi + 1) * RGPIX].rearrange(
                "p (r c) -> p r c", c=W)
            prod = prodp.tile([128, RGPIX], BF16, tag="prod", bufs=6)
            eng = nc.gpsimd if gi in (3, 7, 11) else nc.vector
            eng.tensor_tensor(prod[:, :].rearrange("p (r c) -> p r c", c=W),
                              vv, ebv, AL.mult)
            nc.tensor.matmul(ps_red[:, :], t_sel4[:], prod[:, :],
                             start=(gi == 0), stop=(gi == NG - 1))
        dtile, doff = (out_all[0], 32 * h) if h < 4 else (out_all[1], 32 * (h - 4))
        nc.scalar.activation(dtile[doff:doff + 32, n0:n0 + RGPIX],
                             ps_red[0:32, :], AF.Identity)

    pending = None
    for h in range(HEADS):
        e_h = epool.tile([E56, PIXI], BF16, tag="e")
        for rg in range(RG):
            n0 = rg * RGPIX
            ps_att = patt.tile([128, RGPIX], F32, tag="att")
            qv = qkp[h][:].rearrange("p (r c) -> p r c", c=RW)
            first = True
            for dy in range(WIN):
                for j in range(4):
                    nc.tensor.matmul(
                        ps_att[:, :],
                        w_dw[:, (dy * 4 + j) * 128:(dy * 4 + j + 1) * 128],
                        qv[:, rg * RGROWS + dy:rg * RGROWS + dy + RGROWS,
                           2 * j:2 * j + W],
                        start=first, stop=(dy == WIN - 1 and j == 3))
                    first = False
            nc.scalar.activation(e_h[:, n0:n0 + RGPIX], ps_att[0:E56, :],
                                 AF.Exp, bias=t_bias6[:, h:h + 1])
            i24 = 4 * h + rg
            nc.tensor.matmul(ps_zall[:, :],
                             t_zsel[:, 128 * i24:128 * i24 + 128],
                             e_h[:, n0:n0 + RGPIX],
                             start=(i24 == 0), stop=(i24 == 23))
            # broadcast e rows to 4x32 partition blocks: stage the rg slice
            # to DRAM (flat, no partition-step limits), then 4 DMAs read it
            # back 32x-replicated (dst block g=2dy+half, partitions 32j get
            # row 4g+j; rows 4g+3 of odd g are the zero pad rows)
            e_dram = dram.tile([E56, RGPIX], BF16, tag="edram", bufs=2)
            nc.sync.dma_start(e_dram[:, :], e_h[:, n0:n0 + RGPIX])
            ebc = ebcp.tile([128, NG * RGPIX], BF16, tag="ebc")
            for j in range(4):
                src = (e_dram[:, :]
                       .rearrange("(g f) n -> g f n", f=4)[:, j, :]
                       .partition_broadcast(32))
                dst = ebc[32 * j:32 * j + 32, :].rearrange(
                    "r (g n) -> r g n", n=RGPIX)
                qeng = nc.sync if j < 2 else nc.gpsimd
                qeng.dma_start(dst, src)
            if pending is not None:
                emit_agg(**pending)
            pending = dict(h=h, rg=rg, ebc=ebc, n0=n0)
    emit_agg(**pending)

    # normalize: out_all *= 1/z broadcast per head to its 32 channels
    nc.vector.reciprocal(rz_all[:, :], ps_zall[0:24, :])
    for nt in range(RG):
        n0 = nt * RGPIX
        for mt in range(2):
            msz = 128 if mt == 0 else 64
            ps_rz = pm.tile([128, 448], F32, tag="pm")
            nc.tensor.matmul(ps_rz[:, 0:RGPIX],
                             t_sel24[:, nt * 256 + 128 * mt:
                                     nt * 256 + 128 * mt + 128],
                             rz_all[:, :], start=True, stop=True)
            nc.vector.tensor_tensor(out_all[mt][0:msz, n0:n0 + RGPIX],
                                    out_all[mt][0:msz, n0:n0 + RGPIX],
                                    ps_rz[0:msz, 0:RGPIX], AL.mult)

    # ---- proj + residual ------------------------------------------------
    x1 = [qkbuf.tile([128, PIXI], BF16, name="x10", tag="qkp2"),
          qkbuf.tile([64, PIXI], BF16, name="x11", tag="qkp3")]
    for nt in range(RG):
        n0 = nt * RGPIX
        for mt in range(2):
            msz = 128 if mt == 0 else 64
            ps = pm.tile([128, 448], F32, tag="pm")
            for kc in range(2):
                ksz = 128 if kc == 0 else 64
                nc.tensor.matmul(ps[0:msz, 0:RGPIX],
                                 w_proj[kc][0:ksz, 128 * mt:128 * mt + msz],
                                 out_all[kc][0:ksz, n0:n0 + RGPIX],
                                 start=(kc == 0), stop=(kc == 1))
            nc.scalar.activation(x1[mt][0:msz, n0:n0 + RGPIX], ps[0:msz, 0:RGPIX],
                                 AF.Identity, bias=t_projb[mt][:])
            xsrc = xa if mt == 0 else xb
            nc.gpsimd.tensor_tensor(
                x1[mt][0:msz, n0:n0 + RGPIX], x1[mt][0:msz, n0:n0 + RGPIX],
                xsrc[0:msz, i0 + n0:i0 + n0 + RGPIX], AL.add)

    # ---- MLP ------------------------------------------------------------
    hten = [vbuf.tile([128, PIXI], BF16, name=f"h{i}", tag=f"vrep{i}")
            for i in range(6)]
    for nt in range(RG):
        n0 = nt * RGPIX
        for mt in range(6):
            ps = pm.tile([128, 448], F32, tag="pm")
            for kc in range(2):
                ksz = 128 if kc == 0 else 64
                nc.tensor.matmul(ps[:, 0:RGPIX],
                                 w_c1[kc][0:ksz, 128 * mt:128 * (mt + 1)],
                                 x1[kc][0:ksz, n0:n0 + RGPIX],
                                 start=(kc == 0), stop=(kc == 1))
            nc.scalar.activation(hten[mt][:, n0:n0 + RGPIX], ps[:, 0:RGPIX],
                                 AF.Gelu, bias=t_c1b[mt][:])
    for nt in range(RG):
        n0 = nt * RGPIX
        for mt in range(2):
            msz = 128 if mt == 0 else 64
            ps = pm.tile([128, 448], F32, tag="pm")
            for kc in range(6):
                nc.tensor.matmul(ps[0:msz, 0:RGPIX],
                                 w_c2[kc][:, 128 * mt:128 * mt + msz],
                                 hten[kc][:, n0:n0 + RGPIX],
                                 start=(kc == 0), stop=(kc == 5))
            of = prodp.tile([128, RGPIX], F32, tag="of", bufs=2)
            nc.scalar.activation(of[0:msz, :], ps[0:msz, 0:RGPIX],
                                 AF.Identity, bias=t_c2b[mt][:])
            nc.gpsimd.tensor_tensor(of[0:msz, :], of[0:msz, :],
                                    x1[mt][0:msz, n0:n0 + RGPIX], AL.add)
            nc.sync.dma_start(out[128 * mt:128 * mt + msz, n0:n0 + RGPIX],
                              of[0:msz, :])


_PROGRAM = None


def _get_program():
    global _PROGRAM
    if _PROGRAM is None:
        _PROGRAM = build_program()
    return _PROGRAM


def make_in_maps(inputs):
    consts = _host_consts(inputs)
    xs = _x_slices(np.asarray(inputs["x"], np.float32))
    return [{"xl": xs[c], **consts} for c in range(NCORES)]


def assemble(results):
    out = np.empty((B, DIM, H, W), np.float32)
    for c in range(NCORES):
        b, yh = c // 2, c % 2
        y0 = ROWS * yh
        out[b, :, y0:y0 + ROWS, :] = results[c]["out"].reshape(DIM, ROWS, W)
    return out


def kernel(**inputs) -> np.ndarray:
    nc = _get_program()
    in_maps = make_in_maps(inputs)
    res = run_bass_kernel_spmd(nc, in_maps, list(range(NCORES)))
    return assemble(res.results)


if __name__ == "__main__":
    import reference
    inp = {k: np.asarray(v) for k, v in reference.setup_inputs().items()}
    got = kernel(**inp)
    exp = np.asarray(reference.reference(**reference.setup_inputs()))
    err = np.abs(got - exp).max() / np.abs(exp).max()
    print("rel err:", err)

